# revision 10
# baseline (speedup 1.0000x reference)
"""MLA (DeepSeek-style) attention layer on 8 Trainium2 NeuronCores.

Sharding: core c -> batch b = c//4, head group g = c%4 (4 of 16 heads).
Each core computes a partial output (its heads' contribution through its
W_O row-slice); the host sums the 4 partials per batch.

v2: four phases, causal attention.
  P1: c_Q down-proj (rms-scaled in-phase), spilled to DRAM.
  P2: c_KV down-proj fused with k/v up-projection + k-rope (no spill).
  P3: c_Q reload -> q up-projection + q-rope.
  P4: block-causal attention (diag-chunk masks only) + row-parallel W_O.
Weights are loaded once each in single large DMAs; x is streamed twice
(P1/P2) in one DMA per 256-token chunk. DMA issue is spread over the
Pool (SWDGE) and SP/ACT (HWDGE) queues to avoid sequencer serialization.
The q/k path stays fp32r end-to-end for logit accuracy; P/V/W_O run bf16.
"""
import sys

for _p in ("/opt/trn_rl_repo", "/root/.axon_site/_ro/trn_rl_repo"):
    if _p not in sys.path:
        sys.path.append(_p)

import numpy as np
import ml_dtypes

B, S, D = 2, 2048, 2048
H, NOPE, ROPE, VD = 16, 128, 64, 128
DCQ, DCKV = 1536, 512
EPS = 1e-6
SCALE = float(np.sqrt(NOPE + ROPE))
HL = 4           # local heads per core
NCORES = 8
NQT = S // 128   # 16
NKC = S // 512   # 4
NKD = D // 128   # 16
NMQ = DCQ // 128  # 12
NMKV = DCKV // 128  # 4
CH = 256         # token chunk for P1-P3
NCH = S // CH    # 8
BF = ml_dtypes.bfloat16

_BUILD_CACHE = {}


def build_nc():
    import concourse.tile as tile
    import concourse.mybir as mybir
    from concourse import bacc

    F32 = mybir.dt.float32
    F32R = mybir.dt.float32r
    BF16 = mybir.dt.bfloat16

    nc = bacc.Bacc(num_devices=NCORES)

    T = {}
    T["xT"] = nc.dram_tensor("xT", [D, S], F32R, kind="ExternalInput")
    T["maskp"] = nc.dram_tensor("maskp", [512, 512], BF16, kind="ExternalInput")
    T["cos4"] = nc.dram_tensor("cos4", [128, S], F32, kind="ExternalInput")
    T["sin4"] = nc.dram_tensor("sin4", [128, S], F32, kind="ExternalInput")
    T["wdq"] = nc.dram_tensor("wdq", [D, DCQ], F32R, kind="ExternalInput")
    T["wdkv"] = nc.dram_tensor("wdkv", [D, DCKV], F32R, kind="ExternalInput")
    T["wdkr"] = nc.dram_tensor("wdkr", [D, 64], F32R, kind="ExternalInput")
    T["wuq"] = nc.dram_tensor("wuq", [DCQ, HL * NOPE], F32R, kind="ExternalInput")
    T["wuqre"] = nc.dram_tensor("wuqre", [DCQ, HL * 32], F32R, kind="ExternalInput")
    T["wuqro"] = nc.dram_tensor("wuqro", [DCQ, HL * 32], F32R, kind="ExternalInput")
    T["wuk"] = nc.dram_tensor("wuk", [DCKV, HL * NOPE], F32R, kind="ExternalInput")
    T["wuv"] = nc.dram_tensor("wuv", [DCKV, HL * VD], F32R, kind="ExternalInput")
    T["wo4"] = nc.dram_tensor("wo4", [HL * VD, D], BF16, kind="ExternalInput")
    T["ident"] = nc.dram_tensor("ident", [128, 128], BF16, kind="ExternalInput")
    T["ones_r"] = nc.dram_tensor("ones_r", [1, 128], F32R, kind="ExternalInput")
    T["ones_c"] = nc.dram_tensor("ones_c", [128, 1], F32R, kind="ExternalInput")
    T["outp"] = nc.dram_tensor("outp", [S, D], F32, kind="ExternalOutput")

    with tile.TileContext(nc) as tc:
        _emit(nc, tc, T)
    nc.compile()
    return nc


def _emit(nc, tc, T):
    import concourse.bass as bass
    import concourse.mybir as mybir

    F32 = mybir.dt.float32
    F32R = mybir.dt.float32r
    BF16 = mybir.dt.bfloat16
    AF = mybir.ActivationFunctionType
    AX = mybir.AxisListType
    ts = bass.ts

    xT, maskp, cos4, sin4 = T["xT"], T["maskp"], T["cos4"], T["sin4"]
    wdq, wdkv, wdkr = T["wdq"], T["wdkv"], T["wdkr"]
    wuq, wuqre, wuqro, wuk, wuv, wo4 = (
        T["wuq"], T["wuqre"], T["wuqro"], T["wuk"], T["wuv"], T["wo4"])
    ident, ones_r, ones_c, outp = T["ident"], T["ones_r"], T["ones_c"], T["outp"]

    xTr = xT.rearrange("(kt p) s -> p kt s", p=128)

    # --- persistent-scope pools, opened in lifetime (LIFO) order ---
    const_p = tc.tile_pool(name="constp", bufs=1)
    const = const_p.__enter__()
    onesr_t = const.tile([1, 128], F32R, tag="onesr")
    nc.sync.dma_start(onesr_t[:], ones_r[:])
    onesc_t = const.tile([128, 1], F32R, tag="onesc")
    nc.sync.dma_start(onesc_t[:], ones_c[:])
    ident_t = const.tile([128, 128], BF16, tag="ident")
    nc.sync.dma_start(ident_t[:], ident[:])
    epst = const.tile([1, 1], F32, tag="epst")
    nc.gpsimd.memset(epst[:], EPS)

    dram_p = tc.tile_pool(name="dram", bufs=1, space="DRAM")
    dram = dram_p.__enter__()
    cqd = dram.tile([128, NMQ, S], F32R, tag="cqd")

    # ============ P1: c_Q down-projection, rms-scaled, spilled ============
    with tc.tile_pool(name="wP1", bufs=1) as wp1, \
         tc.tile_pool(name="xP1", bufs=2) as xp1, \
         tc.tile_pool(name="evP1", bufs=2) as evp1, \
         tc.tile_pool(name="sqP1", bufs=2) as sqp1, \
         tc.tile_pool(name="invP1", bufs=2) as invp1, \
         tc.tile_pool(name="psP1", bufs=3, space="PSUM") as psp1, \
         tc.tile_pool(name="psS1", bufs=2, space="PSUM") as pss1:
        wdq_t = wp1.tile([128, NKD, DCQ], F32R, tag="wdq")
        wdqr = wdq.rearrange("(kt p) m -> p kt m", p=128)
        for k in range(NKD):
            nc.sync.dma_start(wdq_t[:, k, :], wdqr[:, k, :])
        for ch in range(NCH):
            sl = slice(ch * CH, (ch + 1) * CH)
            xq = xp1.tile([128, NKD, CH], F32R, tag="xq", name=f"xq{ch}")
            nc.gpsimd.dma_start(xq[:], xTr[:, :, sl])
            ev = evp1.tile([128, NMQ, CH], F32R, tag="ev", name=f"ev{ch}")
            sum_ps = pss1.tile([1, CH], F32, tag="sum", name=f"sum{ch}")
            for m in range(NMQ):
                ps = psp1.tile([128, CH], F32, tag="dp", name=f"dp{ch}_{m}")
                for k in range(NKD):
                    nc.tensor.matmul(ps[:], wdq_t[:, k, ts(m, 128)], xq[:, k, :],
                                     start=(k == 0), stop=(k == NKD - 1))
                nc.scalar.activation(ev[:, m, :], ps[:], AF.Copy)
                sq = sqp1.tile([128, CH], F32R, tag="sq", name=f"sq{ch}_{m}")
                nc.scalar.activation(sq[:], ps[:], AF.Square)
                nc.tensor.matmul(sum_ps[:], onesc_t[:], sq[:],
                                 start=(m == 0), stop=(m == NMQ - 1))
            rms = invp1.tile([1, CH], F32, tag="rms", name=f"rms{ch}")
            nc.scalar.activation(rms[:], sum_ps[:], AF.Sqrt, bias=epst[:],
                                 scale=1.0 / DCQ)
            inv = invp1.tile([1, CH], F32R, tag="inv", name=f"inv{ch}")
            with nc.allow_low_precision(reason="f32r shares f32 bits"):
                nc.vector.reciprocal(inv[:], rms[:])
            psb = pss1.tile([128, CH], F32, tag="bc", name=f"bc{ch}")
            nc.tensor.matmul(psb[:], onesr_t[:], inv[:], start=True, stop=True)
            invbc = invp1.tile([128, CH], F32R, tag="invbc", name=f"invbc{ch}")
            nc.scalar.activation(invbc[:], psb[:], AF.Copy)
            for m in range(NMQ):
                nc.vector.tensor_mul(ev[:, m, :], ev[:, m, :], invbc[:])
            nc.sync.dma_start(cqd[:, :, sl], ev[:])

    # ============ P2: c_KV down-proj fused with k/v up-proj + k-rope ======
    kside_p = tc.tile_pool(name="kside", bufs=1)
    kside = kside_p.__enter__()
    kT = [kside.tile([128, S], F32R, tag=f"kT{h}", name=f"kT{h}") for h in range(HL)]
    krope2 = kside.tile([128, S], F32R, tag="krope2")
    v_all = kside.tile([128, NQT, HL * VD], BF16, tag="v_all")

    with tc.tile_pool(name="wP2", bufs=1) as wp2, \
         tc.tile_pool(name="xP2", bufs=2) as xp2, \
         tc.tile_pool(name="evP2", bufs=2) as evp2, \
         tc.tile_pool(name="sqP2", bufs=2) as sqp2, \
         tc.tile_pool(name="invP2", bufs=2) as invp2, \
         tc.tile_pool(name="ropeP2", bufs=2) as rope2, \
         tc.tile_pool(name="psP2", bufs=2, space="PSUM") as psp2, \
         tc.tile_pool(name="psS2", bufs=1, space="PSUM") as pss2, \
         tc.tile_pool(name="psU2", bufs=2, space="PSUM") as psu2:
        wdkv_t = wp2.tile([128, NKD, DCKV], F32R, tag="wdkv")
        wdkvr = wdkv.rearrange("(kt p) m -> p kt m", p=128)
        for k in range(NKD):
            nc.sync.dma_start(wdkv_t[:, k, :], wdkvr[:, k, :])
        wdkr_t = wp2.tile([128, NKD, 64], F32R, tag="wdkr")
        nc.sync.dma_start(wdkr_t[:], wdkr.rearrange("(kt p) m -> p kt m", p=128))
        wuk_t = wp2.tile([128, NMKV, HL * NOPE], F32R, tag="wuk")
        wukr = wuk.rearrange("(kt p) m -> p kt m", p=128)
        wuv_t = wp2.tile([128, NMKV, HL * VD], F32R, tag="wuv")
        wuvr = wuv.rearrange("(kt p) m -> p kt m", p=128)
        for k in range(NMKV):
            nc.sync.dma_start(wuk_t[:, k, :], wukr[:, k, :])
            nc.sync.dma_start(wuv_t[:, k, :], wuvr[:, k, :])
        for ch in range(NCH):
            sl = slice(ch * CH, (ch + 1) * CH)
            xq = xp2.tile([128, NKD, CH], F32R, tag="xq", name=f"x2{ch}")
            nc.gpsimd.dma_start(xq[:], xTr[:, :, sl])
            ev = evp2.tile([128, NMKV, CH], F32R, tag="ev", name=f"e2{ch}")
            sum_ps = pss2.tile([1, CH], F32, tag="sum", name=f"s2{ch}")
            for m in range(NMKV):
                ps = psp2.tile([128, CH], F32, tag="dp", name=f"d2{ch}_{m}")
                for k in range(NKD):
                    nc.tensor.matmul(ps[:], wdkv_t[:, k, ts(m, 128)], xq[:, k, :],
                                     start=(k == 0), stop=(k == NKD - 1))
                nc.scalar.activation(ev[:, m, :], ps[:], AF.Copy)
                sq = sqp2.tile([128, CH], F32R, tag="sq", name=f"q2{ch}_{m}")
                nc.scalar.activation(sq[:], ps[:], AF.Square)
                nc.tensor.matmul(sum_ps[:], onesc_t[:], sq[:],
                                 start=(m == 0), stop=(m == NMKV - 1))
            # k_R = x @ W_DKR (no rms norm), rope'd below
            pskr = psp2.tile([128, CH], F32, tag="dp", name=f"kr{ch}")
            for k in range(NKD):
                nc.tensor.matmul(pskr[:64, :], wdkr_t[:, k, :], xq[:, k, :],
                                 start=(k == 0), stop=(k == NKD - 1))
            kre = rope2.tile([32, CH], F32, tag="kre", name=f"kre{ch}")
            nc.scalar.activation(kre[:], pskr[0:32, :], AF.Copy)
            kro = rope2.tile([32, CH], F32, tag="kro", name=f"kro{ch}")
            nc.scalar.activation(kro[:], pskr[32:64, :], AF.Copy)
            rms = invp2.tile([1, CH], F32, tag="rms", name=f"r2{ch}")
            nc.scalar.activation(rms[:], sum_ps[:], AF.Sqrt, bias=epst[:],
                                 scale=1.0 / DCKV)
            inv = invp2.tile([1, CH], F32R, tag="inv", name=f"i2{ch}")
            with nc.allow_low_precision(reason="f32r shares f32 bits"):
                nc.vector.reciprocal(inv[:], rms[:])
            psb = pss2.tile([128, CH], F32, tag="bc", name=f"b2{ch}")
            nc.tensor.matmul(psb[:], onesr_t[:], inv[:], start=True, stop=True)
            invbc = invp2.tile([128, CH], F32R, tag="invbc", name=f"ib2{ch}")
            nc.scalar.activation(invbc[:], psb[:], AF.Copy)
            for m in range(NMKV):
                nc.vector.tensor_mul(ev[:, m, :], ev[:, m, :], invbc[:])
            # k-rope for this chunk
            cs_a = rope2.tile([32, CH], F32, tag="cs_a", name=f"cs{ch}")
            nc.gpsimd.dma_start(cs_a[:], cos4[0:32, sl])
            sn_a = rope2.tile([32, CH], F32, tag="sn_a", name=f"sn{ch}")
            nc.gpsimd.dma_start(sn_a[:], sin4[0:32, sl])
            t1k = rope2.tile([32, CH], F32, tag="t1k", bufs=1, name=f"t1k{ch}")
            nc.vector.tensor_mul(t1k[:], kre[:], cs_a[:])
            t2k = rope2.tile([32, CH], F32, tag="t2k", bufs=1, name=f"t2k{ch}")
            nc.vector.tensor_mul(t2k[:], kro[:], sn_a[:])
            ko1 = rope2.tile([32, CH], F32R, tag="ko1", name=f"ko1{ch}")
            nc.vector.tensor_sub(ko1[:], t1k[:], t2k[:])
            t3k = rope2.tile([32, CH], F32, tag="t3k", bufs=1, name=f"t3k{ch}")
            nc.vector.tensor_mul(t3k[:], kre[:], sn_a[:])
            t4k = rope2.tile([32, CH], F32, tag="t4k", bufs=1, name=f"t4k{ch}")
            nc.vector.tensor_mul(t4k[:], kro[:], cs_a[:])
            ko2 = rope2.tile([32, CH], F32R, tag="ko2", name=f"ko2{ch}")
            nc.vector.tensor_add(ko2[:], t3k[:], t4k[:])
            for rep in range(2):
                nc.sync.dma_start(krope2[ts(rep * 2, 32), sl], ko1[:])
                nc.sync.dma_start(krope2[ts(rep * 2 + 1, 32), sl], ko2[:])
            # k up-projection
            for h in range(HL):
                ps = psu2.tile([128, CH], F32, tag="upk", name=f"uk{ch}_{h}")
                for k in range(NMKV):
                    nc.tensor.matmul(ps[:], wuk_t[:, k, ts(h, 128)], ev[:, k, :],
                                     start=(k == 0), stop=(k == NMKV - 1))
                nc.scalar.activation(kT[h][:, sl], ps[:], AF.Copy)
            # v up-projection (token-major output)
            for vm in range(CH // 128):
                m = ch * (CH // 128) + vm
                ps = psu2.tile([128, HL * VD], F32, tag="upv", name=f"uv{ch}_{vm}")
                for k in range(NMKV):
                    nc.tensor.matmul(ps[:], ev[:, k, ts(vm, 128)], wuv_t[:, k, :],
                                     start=(k == 0), stop=(k == NMKV - 1))
                nc.scalar.activation(v_all[:, m, :], ps[:], AF.Copy)

    # ============ P3: q up-projection + q-rope ============
    qside_p = tc.tile_pool(name="qside", bufs=1)
    qside = qside_p.__enter__()
    qT = [qside.tile([128, S], F32R, tag=f"qT{h}", name=f"qT{h}") for h in range(HL)]
    qrope = [qside.tile([128, S], F32R, tag=f"qrope{p}", name=f"qrope{p}")
             for p in range(2)]

    with tc.tile_pool(name="wP3", bufs=1) as wp3, \
         tc.tile_pool(name="cqP3", bufs=2) as cqp3, \
         tc.tile_pool(name="csP3", bufs=2) as csp3, \
         tc.tile_pool(name="ropeP3", bufs=2) as rope3, \
         tc.tile_pool(name="psP3", bufs=3, space="PSUM") as psp3:
        wuq_t = wp3.tile([128, NMQ, HL * NOPE], F32R, tag="wuq")
        wuqr_ = wuq.rearrange("(kt p) m -> p kt m", p=128)
        wuqre_t = wp3.tile([128, NMQ, HL * 32], F32R, tag="wuqre")
        wuqrer = wuqre.rearrange("(kt p) m -> p kt m", p=128)
        wuqro_t = wp3.tile([128, NMQ, HL * 32], F32R, tag="wuqro")
        wuqror = wuqro.rearrange("(kt p) m -> p kt m", p=128)
        for k in range(NMQ):
            nc.sync.dma_start(wuq_t[:, k, :], wuqr_[:, k, :])
            nc.sync.dma_start(wuqre_t[:, k, :], wuqrer[:, k, :])
            nc.sync.dma_start(wuqro_t[:, k, :], wuqror[:, k, :])
        for ch in range(NCH):
            sl = slice(ch * CH, (ch + 1) * CH)
            cq = cqp3.tile([128, NMQ, CH], F32R, tag="cq", name=f"cq{ch}")
            nc.gpsimd.dma_start(cq[:], cqd[:, :, sl])
            cos_t = csp3.tile([128, CH], F32, tag="cos", name=f"cos{ch}")
            nc.gpsimd.dma_start(cos_t[:], cos4[:, sl])
            sin_t = csp3.tile([128, CH], F32, tag="sin", name=f"sin{ch}")
            nc.gpsimd.dma_start(sin_t[:], sin4[:, sl])
            for h in range(HL):
                ps = psp3.tile([128, CH], F32, tag="up", name=f"uq{ch}_{h}")
                for k in range(NMQ):
                    nc.tensor.matmul(ps[:], wuq_t[:, k, ts(h, 128)], cq[:, k, :],
                                     start=(k == 0), stop=(k == NMQ - 1))
                nc.scalar.activation(qT[h][:, sl], ps[:], AF.Copy)
            psE = psp3.tile([128, CH], F32, tag="up", name=f"ue{ch}")
            for k in range(NMQ):
                nc.tensor.matmul(psE[:], wuqre_t[:, k, :], cq[:, k, :],
                                 start=(k == 0), stop=(k == NMQ - 1))
            esc = rope3.tile([128, CH], F32, tag="esc", bufs=1, name=f"esc{ch}")
            nc.scalar.activation(esc[:], psE[:], AF.Copy)
            psO = psp3.tile([128, CH], F32, tag="up", name=f"uo{ch}")
            for k in range(NMQ):
                nc.tensor.matmul(psO[:], wuqro_t[:, k, :], cq[:, k, :],
                                 start=(k == 0), stop=(k == NMQ - 1))
            osc = rope3.tile([128, CH], F32, tag="osc", bufs=1, name=f"osc{ch}")
            nc.scalar.activation(osc[:], psO[:], AF.Copy)
            t1 = rope3.tile([128, CH], F32, tag="t1", bufs=1, name=f"t1{ch}")
            nc.vector.tensor_mul(t1[:], esc[:], cos_t[:])
            t2 = rope3.tile([128, CH], F32, tag="t2", bufs=1, name=f"t2{ch}")
            nc.vector.tensor_mul(t2[:], osc[:], sin_t[:])
            o1 = rope3.tile([128, CH], F32R, tag="o1", name=f"o1{ch}")
            nc.vector.tensor_sub(o1[:], t1[:], t2[:])
            t3 = rope3.tile([128, CH], F32, tag="t1", bufs=1, name=f"t3{ch}")
            nc.vector.tensor_mul(t3[:], esc[:], sin_t[:])
            t4 = rope3.tile([128, CH], F32, tag="t2", bufs=1, name=f"t4{ch}")
            nc.vector.tensor_mul(t4[:], osc[:], cos_t[:])
            o2 = rope3.tile([128, CH], F32R, tag="o2", name=f"o2{ch}")
            nc.vector.tensor_add(o2[:], t3[:], t4[:])
            for h in range(HL):
                p, off = h // 2, (h % 2) * 64
                nc.sync.dma_start(qrope[p][off:off + 32, sl], o1[ts(h, 32), :])
                nc.sync.dma_start(qrope[p][off + 32:off + 64, sl],
                                    o2[ts(h, 32), :])

    # ============ P4: block-causal attention + W_O ============
    with tc.tile_pool(name="wo", bufs=1) as wop, \
         tc.tile_pool(name="maskP", bufs=1) as maskpl, \
         tc.tile_pool(name="pu", bufs=2) as pup, \
         tc.tile_pool(name="pT", bufs=2) as pTp, \
         tc.tile_pool(name="attP", bufs=1) as attp, \
         tc.tile_pool(name="osb", bufs=2) as osb, \
         tc.tile_pool(name="stats", bufs=4) as stats, \
         tc.tile_pool(name="psS", bufs=6, space="PSUM") as psS, \
         tc.tile_pool(name="psAV", bufs=1, space="PSUM") as psAV, \
         tc.tile_pool(name="psWO", bufs=1, space="PSUM") as psWO:
        wo_t = wop.tile([128, HL, D], BF16, tag="wo")
        nc.sync.dma_start(wo_t[:], wo4.rearrange("(ht p) m -> p ht m", p=128))
        mts = []
        for qt in range(4):
            mt = maskpl.tile([128, 512], BF16, tag=f"mask{qt}", name=f"mk{qt}")
            nc.sync.dma_start(mt[:], maskp[ts(qt, 128), :])
            mts.append(mt)
        for qb in range(4):
            nv = qb + 1              # valid 512-wide key chunks
            vw = nv * 512            # valid key width
            att = [None] * HL
            pTs = [None] * HL

            def do_av(h):
                pav = psAV.tile([128, 512], F32, tag="av", name=f"av{qb}_{h}")
                for kt in range(4 * nv):
                    nc.tensor.matmul(pav[:], v_all[:, kt, ts(h, 128)],
                                     pTs[h][:, kt, :],
                                     start=(kt == 0), stop=(kt == 4 * nv - 1))
                at = attp.tile([128, 512], BF16, tag=f"att{h}", name=f"at{qb}_{h}")
                nc.scalar.activation(at[:], pav[:], AF.Copy)
                att[h] = at

            for h in range(HL):
                pT_t = pTp.tile([128, NQT, 512], BF16, tag="pT", name=f"pT{qb}_{h}")
                pTs[h] = pT_t
                for qt in range(4):
                    qsl = slice((qb * 4 + qt) * 128, (qb * 4 + qt + 1) * 128)
                    phc = [psS.tile([128, 512], F32, tag="qk",
                                    name=f"qk{qb}_{h}_{qt}_{c}")
                           for c in range(nv)]
                    off = (h % 2) * 64
                    for c in range(nv):
                        pp = phc[c][:]
                        ksl = slice(c * 512, (c + 1) * 512)
                        nc.tensor.matmul(pp, qT[h][:, qsl], kT[h][:, ksl],
                                         start=True, stop=False)
                        nc.tensor.matmul(pp, qrope[h // 2][off:off + 64, qsl],
                                         krope2[off:off + 64, ksl],
                                         start=False, stop=(c != qb))
                        if c == qb:
                            nc.tensor.matmul(pp, ident_t[:], mts[qt][:],
                                             start=False, stop=True)
                    mxs = []
                    for c in range(nv):
                        mx = stats.tile([128, 1], F32, tag=f"mx{c}",
                                        name=f"mx{qb}{h}{qt}_{c}")
                        nc.vector.reduce_max(mx[:], phc[c][:], axis=AX.X)
                        mxs.append(mx)
                    mxc = mxs[0]
                    for c in range(1, nv):
                        t = stats.tile([128, 1], F32, tag=f"cmb{c}",
                                       name=f"cmb{qb}{h}{qt}_{c}")
                        nc.vector.tensor_max(t[:], mxc[:], mxs[c][:])
                        mxc = t
                    negm = stats.tile([128, 1], F32, tag="negm", name=f"ng{qb}{h}{qt}")
                    nc.vector.tensor_scalar_mul(negm[:], mxc[:], -SCALE)
                    pu = pup.tile([128, S], BF16, tag="pu", name=f"pu{qb}{h}{qt}")
                    las = []
                    for c in range(nv):
                        la = stats.tile([128, 1], F32, tag=f"la{c}",
                                        name=f"la{qb}{h}{qt}_{c}")
                        nc.scalar.activation(pu[:, ts(c, 512)], phc[c][:], AF.Exp,
                                             bias=negm[:], scale=SCALE,
                                             accum_out=la[:])
                        las.append(la)
                    lt = las[0]
                    for c in range(1, nv):
                        t2 = stats.tile([128, 1], F32, tag=f"lts{c}",
                                        name=f"lts{qb}{h}{qt}_{c}")
                        nc.vector.tensor_add(t2[:], lt[:], las[c][:])
                        lt = t2
                    rl = stats.tile([128, 1], F32, tag="rl", name=f"rl{qb}{h}{qt}")
                    nc.vector.reciprocal(rl[:], lt[:])
                    nc.vector.tensor_scalar_mul(pu[:, 0:vw], pu[:, 0:vw], rl[:])
                    nc.sync.dma_start(pT_t[:, 0:4 * nv, ts(qt, 128)], pu[:, 0:vw],
                                      transpose=True)
                if h > 0:
                    do_av(h - 1)
            do_av(HL - 1)
            for qt in range(4):
                qrow = (qb * 4 + qt) * 128
                ot = osb.tile([128, D], F32, tag="ot", name=f"ot{qb}{qt}")
                for dch in range(4):
                    pw = psWO.tile([128, 512], F32, tag="wops", name=f"wo{qb}{qt}{dch}")
                    for h in range(HL):
                        nc.tensor.matmul(pw[:], att[h][:, ts(qt, 128)],
                                         wo_t[:, h, ts(dch, 512)],
                                         start=(h == 0), stop=(h == HL - 1))
                    nc.vector.tensor_copy(ot[:, ts(dch, 512)], pw[:])
                nc.gpsimd.dma_start(outp[qrow:qrow + 128, :], ot[:])

    qside_p.__exit__(None, None, None)
    kside_p.__exit__(None, None, None)
    dram_p.__exit__(None, None, None)
    const_p.__exit__(None, None, None)


def _shard(inputs):
    x = np.asarray(inputs["x"], np.float32)
    mask = np.asarray(inputs["mask"], np.float32)[0, 0]
    pos_cos = np.asarray(inputs["pos_cos"], np.float32)
    pos_sin = np.asarray(inputs["pos_sin"], np.float32)
    W_DQ = np.asarray(inputs["W_DQ"], np.float32)
    W_UQ = np.asarray(inputs["W_UQ"], np.float32)
    W_UQR = np.asarray(inputs["W_UQR"], np.float32)
    W_DKV = np.asarray(inputs["W_DKV"], np.float32)
    W_UK = np.asarray(inputs["W_UK"], np.float32)
    W_UV = np.asarray(inputs["W_UV"], np.float32)
    W_DKR = np.asarray(inputs["W_DKR"], np.float32)
    W_O = np.asarray(inputs["W_O"], np.float32)
    qw = np.asarray(inputs["q_norm_w"], np.float32)
    kvw = np.asarray(inputs["kv_norm_w"], np.float32)

    maskp = (mask[0:512, 0:512] / SCALE).astype(BF)
    cos4 = np.tile(np.ascontiguousarray(pos_cos.T), (4, 1)).astype(np.float32)
    sin4 = np.tile(np.ascontiguousarray(pos_sin.T), (4, 1)).astype(np.float32)
    wdkr = np.ascontiguousarray(
        np.concatenate([W_DKR[:, 0::2], W_DKR[:, 1::2]], axis=1))
    wuq_n = W_UQ * qw[:, None]
    wuqr_n = (W_UQR * qw[:, None]).reshape(DCQ, H, ROPE)
    wuk_n = W_UK * kvw[:, None]
    wuv_n = W_UV * kvw[:, None]
    ident = np.eye(128, dtype=np.float32).astype(BF)
    ones_r = np.ones((1, 128), np.float32)
    ones_c = np.ones((128, 1), np.float32)

    in_maps = []
    for c in range(NCORES):
        b, g = divmod(c, 4)
        hs = slice(g * HL * NOPE, (g + 1) * HL * NOPE)
        heads = list(range(g * HL, (g + 1) * HL))
        wuqre = np.concatenate([wuqr_n[:, h, 0::2] for h in heads], axis=1)
        wuqro = np.concatenate([wuqr_n[:, h, 1::2] for h in heads], axis=1)
        in_maps.append({
            "xT": np.ascontiguousarray(x[b].T),
            "maskp": maskp,
            "cos4": cos4,
            "sin4": sin4,
            "wdq": W_DQ,
            "wdkv": W_DKV,
            "wdkr": wdkr,
            "wuq": np.ascontiguousarray(wuq_n[:, hs]),
            "wuqre": np.ascontiguousarray(wuqre),
            "wuqro": np.ascontiguousarray(wuqro),
            "wuk": np.ascontiguousarray(wuk_n[:, hs]),
            "wuv": np.ascontiguousarray(wuv_n[:, hs]),
            "wo4": np.ascontiguousarray(W_O[hs, :]).astype(BF),
            "ident": ident,
            "ones_r": ones_r,
            "ones_c": ones_c,
        })
    return in_maps


def kernel(**inputs):
    from concourse.bass_utils import run_bass_kernel_spmd

    if "nc" not in _BUILD_CACHE:
        _BUILD_CACHE["nc"] = build_nc()
    nc = _BUILD_CACHE["nc"]
    in_maps = _shard(inputs)
    res = run_bass_kernel_spmd(nc, in_maps, core_ids=list(range(NCORES)))
    out = np.zeros((B, S, D), np.float32)
    for c in range(NCORES):
        out[c // 4] += np.asarray(res.results[c]["outp"], np.float32)
    return out


# revision 11
# speedup vs baseline: 1.0034x; 1.0034x over previous
"""MLA (DeepSeek-style) attention layer on 8 Trainium2 NeuronCores.

Sharding: core c -> batch b = c//4, head group g = c%4 (4 of 16 heads).
Each core computes a partial output (its heads' contribution through its
W_O row-slice); the host sums the 4 partials per batch.

v2: four phases, causal attention.
  P1: c_Q down-proj (rms-scaled in-phase), spilled to DRAM.
  P2: c_KV down-proj fused with k/v up-projection + k-rope (no spill).
  P3: c_Q reload -> q up-projection + q-rope.
  P4: block-causal attention (diag-chunk masks only) + row-parallel W_O.
Weights are loaded once each in single large DMAs; x is streamed twice
(P1/P2) in one DMA per 256-token chunk. DMA issue is spread over the
Pool (SWDGE) and SP/ACT (HWDGE) queues to avoid sequencer serialization.
The q/k path stays fp32r end-to-end for logit accuracy; P/V/W_O run bf16.
"""
import sys

for _p in ("/opt/trn_rl_repo", "/root/.axon_site/_ro/trn_rl_repo"):
    if _p not in sys.path:
        sys.path.append(_p)

import numpy as np
import ml_dtypes

B, S, D = 2, 2048, 2048
H, NOPE, ROPE, VD = 16, 128, 64, 128
DCQ, DCKV = 1536, 512
EPS = 1e-6
SCALE = float(np.sqrt(NOPE + ROPE))
HL = 4           # local heads per core
NCORES = 8
NQT = S // 128   # 16
NKC = S // 512   # 4
NKD = D // 128   # 16
NMQ = DCQ // 128  # 12
NMKV = DCKV // 128  # 4
CH = 256         # token chunk for P1-P3
NCH = S // CH    # 8
BF = ml_dtypes.bfloat16

_BUILD_CACHE = {}


def build_nc():
    import concourse.tile as tile
    import concourse.mybir as mybir
    from concourse import bacc

    F32 = mybir.dt.float32
    F32R = mybir.dt.float32r
    BF16 = mybir.dt.bfloat16

    nc = bacc.Bacc(num_devices=NCORES)

    T = {}
    T["xT"] = nc.dram_tensor("xT", [D, S], F32R, kind="ExternalInput")
    T["maskp"] = nc.dram_tensor("maskp", [512, 512], BF16, kind="ExternalInput")
    T["cos4"] = nc.dram_tensor("cos4", [128, S], F32, kind="ExternalInput")
    T["sin4"] = nc.dram_tensor("sin4", [128, S], F32, kind="ExternalInput")
    T["wdq"] = nc.dram_tensor("wdq", [D, DCQ], F32R, kind="ExternalInput")
    T["wdkv"] = nc.dram_tensor("wdkv", [D, DCKV], F32R, kind="ExternalInput")
    T["wdkr"] = nc.dram_tensor("wdkr", [D, 64], F32R, kind="ExternalInput")
    T["wuq"] = nc.dram_tensor("wuq", [DCQ, HL * NOPE], F32R, kind="ExternalInput")
    T["wuqre"] = nc.dram_tensor("wuqre", [DCQ, HL * 32], F32R, kind="ExternalInput")
    T["wuqro"] = nc.dram_tensor("wuqro", [DCQ, HL * 32], F32R, kind="ExternalInput")
    T["wuk"] = nc.dram_tensor("wuk", [DCKV, HL * NOPE], F32R, kind="ExternalInput")
    T["wuv"] = nc.dram_tensor("wuv", [DCKV, HL * VD], F32R, kind="ExternalInput")
    T["wo4"] = nc.dram_tensor("wo4", [HL * VD, D], BF16, kind="ExternalInput")
    T["ident"] = nc.dram_tensor("ident", [128, 128], BF16, kind="ExternalInput")
    T["ones_r"] = nc.dram_tensor("ones_r", [1, 128], F32R, kind="ExternalInput")
    T["ones_c"] = nc.dram_tensor("ones_c", [128, 1], F32R, kind="ExternalInput")
    T["outp"] = nc.dram_tensor("outp", [S, D], F32, kind="ExternalOutput")

    with tile.TileContext(nc) as tc:
        _emit(nc, tc, T)
    nc.compile()
    return nc


def _emit(nc, tc, T):
    import concourse.bass as bass
    import concourse.mybir as mybir

    F32 = mybir.dt.float32
    F32R = mybir.dt.float32r
    BF16 = mybir.dt.bfloat16
    AF = mybir.ActivationFunctionType
    AX = mybir.AxisListType
    ts = bass.ts

    xT, maskp, cos4, sin4 = T["xT"], T["maskp"], T["cos4"], T["sin4"]
    wdq, wdkv, wdkr = T["wdq"], T["wdkv"], T["wdkr"]
    wuq, wuqre, wuqro, wuk, wuv, wo4 = (
        T["wuq"], T["wuqre"], T["wuqro"], T["wuk"], T["wuv"], T["wo4"])
    ident, ones_r, ones_c, outp = T["ident"], T["ones_r"], T["ones_c"], T["outp"]

    xTr = xT.rearrange("(kt p) s -> p kt s", p=128)

    # --- persistent-scope pools, opened in lifetime (LIFO) order ---
    const_p = tc.tile_pool(name="constp", bufs=1)
    const = const_p.__enter__()
    onesr_t = const.tile([1, 128], F32R, tag="onesr")
    nc.sync.dma_start(onesr_t[:], ones_r[:])
    onesc_t = const.tile([128, 1], F32R, tag="onesc")
    nc.sync.dma_start(onesc_t[:], ones_c[:])
    ident_t = const.tile([128, 128], BF16, tag="ident")
    nc.sync.dma_start(ident_t[:], ident[:])
    epst = const.tile([1, 1], F32, tag="epst")
    nc.gpsimd.memset(epst[:], EPS)

    dram_p = tc.tile_pool(name="dram", bufs=1, space="DRAM")
    dram = dram_p.__enter__()
    cqd = dram.tile([128, NMQ, S], F32R, tag="cqd")

    # ============ P1: c_Q down-projection, rms-scaled, spilled ============
    with tc.tile_pool(name="wP1", bufs=1) as wp1, \
         tc.tile_pool(name="xP1", bufs=2) as xp1, \
         tc.tile_pool(name="evP1", bufs=2) as evp1, \
         tc.tile_pool(name="sqP1", bufs=2) as sqp1, \
         tc.tile_pool(name="invP1", bufs=2) as invp1, \
         tc.tile_pool(name="psP1", bufs=3, space="PSUM") as psp1, \
         tc.tile_pool(name="psS1", bufs=2, space="PSUM") as pss1:
        wdq_t = wp1.tile([128, NKD, DCQ], F32R, tag="wdq")
        wdqr = wdq.rearrange("(kt p) m -> p kt m", p=128)
        for k in range(NKD):
            nc.sync.dma_start(wdq_t[:, k, :], wdqr[:, k, :])
        for ch in range(NCH):
            sl = slice(ch * CH, (ch + 1) * CH)
            xq = xp1.tile([128, NKD, CH], F32R, tag="xq", name=f"xq{ch}")
            nc.gpsimd.dma_start(xq[:], xTr[:, :, sl])
            ev = evp1.tile([128, NMQ, CH], F32R, tag="ev", name=f"ev{ch}")
            sum_ps = pss1.tile([1, CH], F32, tag="sum", name=f"sum{ch}")
            for m in range(NMQ):
                ps = psp1.tile([128, CH], F32, tag="dp", name=f"dp{ch}_{m}")
                for k in range(NKD):
                    nc.tensor.matmul(ps[:], wdq_t[:, k, ts(m, 128)], xq[:, k, :],
                                     start=(k == 0), stop=(k == NKD - 1))
                nc.scalar.activation(ev[:, m, :], ps[:], AF.Copy)
                sq = sqp1.tile([128, CH], F32R, tag="sq", name=f"sq{ch}_{m}")
                nc.scalar.activation(sq[:], ps[:], AF.Square)
                nc.tensor.matmul(sum_ps[:], onesc_t[:], sq[:],
                                 start=(m == 0), stop=(m == NMQ - 1))
            rms = invp1.tile([1, CH], F32, tag="rms", name=f"rms{ch}")
            nc.scalar.activation(rms[:], sum_ps[:], AF.Sqrt, bias=epst[:],
                                 scale=1.0 / DCQ)
            inv = invp1.tile([1, CH], F32R, tag="inv", name=f"inv{ch}")
            with nc.allow_low_precision(reason="f32r shares f32 bits"):
                nc.vector.reciprocal(inv[:], rms[:])
            psb = pss1.tile([128, CH], F32, tag="bc", name=f"bc{ch}")
            nc.tensor.matmul(psb[:], onesr_t[:], inv[:], start=True, stop=True)
            invbc = invp1.tile([128, CH], F32R, tag="invbc", name=f"invbc{ch}")
            nc.scalar.activation(invbc[:], psb[:], AF.Copy)
            for m in range(NMQ):
                nc.vector.tensor_mul(ev[:, m, :], ev[:, m, :], invbc[:])
            nc.sync.dma_start(cqd[:, :, sl], ev[:])

    # ============ P2: c_KV down-proj fused with k/v up-proj + k-rope ======
    kside_p = tc.tile_pool(name="kside", bufs=1)
    kside = kside_p.__enter__()
    kT = [kside.tile([128, S], F32R, tag=f"kT{h}", name=f"kT{h}") for h in range(HL)]
    krope2 = kside.tile([128, S], F32R, tag="krope2")
    v_all = kside.tile([128, NQT, HL * VD], BF16, tag="v_all")

    with tc.tile_pool(name="wP2", bufs=1) as wp2, \
         tc.tile_pool(name="xP2", bufs=2) as xp2, \
         tc.tile_pool(name="evP2", bufs=2) as evp2, \
         tc.tile_pool(name="sqP2", bufs=2) as sqp2, \
         tc.tile_pool(name="invP2", bufs=2) as invp2, \
         tc.tile_pool(name="ropeP2", bufs=2) as rope2, \
         tc.tile_pool(name="psP2", bufs=2, space="PSUM") as psp2, \
         tc.tile_pool(name="psS2", bufs=1, space="PSUM") as pss2, \
         tc.tile_pool(name="psU2", bufs=2, space="PSUM") as psu2:
        wdkv_t = wp2.tile([128, NKD, DCKV], F32R, tag="wdkv")
        wdkvr = wdkv.rearrange("(kt p) m -> p kt m", p=128)
        for k in range(NKD):
            nc.sync.dma_start(wdkv_t[:, k, :], wdkvr[:, k, :])
        wdkr_t = wp2.tile([128, NKD, 64], F32R, tag="wdkr")
        nc.sync.dma_start(wdkr_t[:], wdkr.rearrange("(kt p) m -> p kt m", p=128))
        wuk_t = wp2.tile([128, NMKV, HL * NOPE], F32R, tag="wuk")
        wukr = wuk.rearrange("(kt p) m -> p kt m", p=128)
        wuv_t = wp2.tile([128, NMKV, HL * VD], F32R, tag="wuv")
        wuvr = wuv.rearrange("(kt p) m -> p kt m", p=128)
        for k in range(NMKV):
            nc.sync.dma_start(wuk_t[:, k, :], wukr[:, k, :])
            nc.sync.dma_start(wuv_t[:, k, :], wuvr[:, k, :])
        for ch in range(NCH):
            sl = slice(ch * CH, (ch + 1) * CH)
            xq = xp2.tile([128, NKD, CH], F32R, tag="xq", name=f"x2{ch}")
            nc.gpsimd.dma_start(xq[:], xTr[:, :, sl])
            ev = evp2.tile([128, NMKV, CH], F32R, tag="ev", name=f"e2{ch}")
            sum_ps = pss2.tile([1, CH], F32, tag="sum", name=f"s2{ch}")
            for m in range(NMKV):
                ps = psp2.tile([128, CH], F32, tag="dp", name=f"d2{ch}_{m}")
                for k in range(NKD):
                    nc.tensor.matmul(ps[:], wdkv_t[:, k, ts(m, 128)], xq[:, k, :],
                                     start=(k == 0), stop=(k == NKD - 1))
                nc.scalar.activation(ev[:, m, :], ps[:], AF.Copy)
                sq = sqp2.tile([128, CH], F32R, tag="sq", name=f"q2{ch}_{m}")
                nc.scalar.activation(sq[:], ps[:], AF.Square)
                nc.tensor.matmul(sum_ps[:], onesc_t[:], sq[:],
                                 start=(m == 0), stop=(m == NMKV - 1))
            # k_R = x @ W_DKR (no rms norm), rope'd below
            pskr = psp2.tile([128, CH], F32, tag="dp", name=f"kr{ch}")
            for k in range(NKD):
                nc.tensor.matmul(pskr[:64, :], wdkr_t[:, k, :], xq[:, k, :],
                                 start=(k == 0), stop=(k == NKD - 1))
            kre = rope2.tile([32, CH], F32, tag="kre", name=f"kre{ch}")
            nc.scalar.activation(kre[:], pskr[0:32, :], AF.Copy)
            kro = rope2.tile([32, CH], F32, tag="kro", name=f"kro{ch}")
            nc.scalar.activation(kro[:], pskr[32:64, :], AF.Copy)
            rms = invp2.tile([1, CH], F32, tag="rms", name=f"r2{ch}")
            nc.scalar.activation(rms[:], sum_ps[:], AF.Sqrt, bias=epst[:],
                                 scale=1.0 / DCKV)
            inv = invp2.tile([1, CH], F32R, tag="inv", name=f"i2{ch}")
            with nc.allow_low_precision(reason="f32r shares f32 bits"):
                nc.vector.reciprocal(inv[:], rms[:])
            psb = pss2.tile([128, CH], F32, tag="bc", name=f"b2{ch}")
            nc.tensor.matmul(psb[:], onesr_t[:], inv[:], start=True, stop=True)
            invbc = invp2.tile([128, CH], F32R, tag="invbc", name=f"ib2{ch}")
            nc.scalar.activation(invbc[:], psb[:], AF.Copy)
            for m in range(NMKV):
                nc.vector.tensor_mul(ev[:, m, :], ev[:, m, :], invbc[:])
            # k-rope for this chunk
            cs_a = rope2.tile([32, CH], F32, tag="cs_a", name=f"cs{ch}")
            nc.gpsimd.dma_start(cs_a[:], cos4[0:32, sl])
            sn_a = rope2.tile([32, CH], F32, tag="sn_a", name=f"sn{ch}")
            nc.gpsimd.dma_start(sn_a[:], sin4[0:32, sl])
            t1k = rope2.tile([32, CH], F32, tag="t1k", bufs=1, name=f"t1k{ch}")
            nc.vector.tensor_mul(t1k[:], kre[:], cs_a[:])
            t2k = rope2.tile([32, CH], F32, tag="t2k", bufs=1, name=f"t2k{ch}")
            nc.vector.tensor_mul(t2k[:], kro[:], sn_a[:])
            ko1 = rope2.tile([32, CH], F32R, tag="ko1", name=f"ko1{ch}")
            nc.vector.tensor_sub(ko1[:], t1k[:], t2k[:])
            t3k = rope2.tile([32, CH], F32, tag="t3k", bufs=1, name=f"t3k{ch}")
            nc.vector.tensor_mul(t3k[:], kre[:], sn_a[:])
            t4k = rope2.tile([32, CH], F32, tag="t4k", bufs=1, name=f"t4k{ch}")
            nc.vector.tensor_mul(t4k[:], kro[:], cs_a[:])
            ko2 = rope2.tile([32, CH], F32R, tag="ko2", name=f"ko2{ch}")
            nc.vector.tensor_add(ko2[:], t3k[:], t4k[:])
            for rep in range(2):
                nc.sync.dma_start(krope2[ts(rep * 2, 32), sl], ko1[:])
                nc.sync.dma_start(krope2[ts(rep * 2 + 1, 32), sl], ko2[:])
            # k up-projection
            for h in range(HL):
                ps = psu2.tile([128, CH], F32, tag="upk", name=f"uk{ch}_{h}")
                for k in range(NMKV):
                    nc.tensor.matmul(ps[:], wuk_t[:, k, ts(h, 128)], ev[:, k, :],
                                     start=(k == 0), stop=(k == NMKV - 1))
                nc.scalar.activation(kT[h][:, sl], ps[:], AF.Copy)
            # v up-projection (token-major output)
            for vm in range(CH // 128):
                m = ch * (CH // 128) + vm
                ps = psu2.tile([128, HL * VD], F32, tag="upv", name=f"uv{ch}_{vm}")
                for k in range(NMKV):
                    nc.tensor.matmul(ps[:], ev[:, k, ts(vm, 128)], wuv_t[:, k, :],
                                     start=(k == 0), stop=(k == NMKV - 1))
                nc.scalar.activation(v_all[:, m, :], ps[:], AF.Copy)

    # ============ P3: q up-projection + q-rope ============
    qside_p = tc.tile_pool(name="qside", bufs=1)
    qside = qside_p.__enter__()
    qT = [qside.tile([128, S], F32R, tag=f"qT{h}", name=f"qT{h}") for h in range(HL)]
    qrope = [qside.tile([128, S], F32R, tag=f"qrope{p}", name=f"qrope{p}")
             for p in range(2)]

    with tc.tile_pool(name="wP3", bufs=1) as wp3, \
         tc.tile_pool(name="cqP3", bufs=2) as cqp3, \
         tc.tile_pool(name="csP3", bufs=2) as csp3, \
         tc.tile_pool(name="ropeP3", bufs=2) as rope3, \
         tc.tile_pool(name="psP3", bufs=3, space="PSUM") as psp3:
        wuq_t = wp3.tile([128, NMQ, HL * NOPE], F32R, tag="wuq")
        wuqr_ = wuq.rearrange("(kt p) m -> p kt m", p=128)
        wuqre_t = wp3.tile([128, NMQ, HL * 32], F32R, tag="wuqre")
        wuqrer = wuqre.rearrange("(kt p) m -> p kt m", p=128)
        wuqro_t = wp3.tile([128, NMQ, HL * 32], F32R, tag="wuqro")
        wuqror = wuqro.rearrange("(kt p) m -> p kt m", p=128)
        for k in range(NMQ):
            nc.sync.dma_start(wuq_t[:, k, :], wuqr_[:, k, :])
            nc.sync.dma_start(wuqre_t[:, k, :], wuqrer[:, k, :])
            nc.sync.dma_start(wuqro_t[:, k, :], wuqror[:, k, :])
        for ch in range(NCH):
            sl = slice(ch * CH, (ch + 1) * CH)
            cq = cqp3.tile([128, NMQ, CH], F32R, tag="cq", name=f"cq{ch}")
            nc.gpsimd.dma_start(cq[:], cqd[:, :, sl])
            cos_t = csp3.tile([128, CH], F32, tag="cos", name=f"cos{ch}")
            nc.gpsimd.dma_start(cos_t[:], cos4[:, sl])
            sin_t = csp3.tile([128, CH], F32, tag="sin", name=f"sin{ch}")
            nc.gpsimd.dma_start(sin_t[:], sin4[:, sl])
            for h in range(HL):
                ps = psp3.tile([128, CH], F32, tag="up", name=f"uq{ch}_{h}")
                for k in range(NMQ):
                    nc.tensor.matmul(ps[:], wuq_t[:, k, ts(h, 128)], cq[:, k, :],
                                     start=(k == 0), stop=(k == NMQ - 1))
                nc.scalar.activation(qT[h][:, sl], ps[:], AF.Copy)
            psE = psp3.tile([128, CH], F32, tag="up", name=f"ue{ch}")
            for k in range(NMQ):
                nc.tensor.matmul(psE[:], wuqre_t[:, k, :], cq[:, k, :],
                                 start=(k == 0), stop=(k == NMQ - 1))
            esc = rope3.tile([128, CH], F32, tag="esc", bufs=1, name=f"esc{ch}")
            nc.scalar.activation(esc[:], psE[:], AF.Copy)
            psO = psp3.tile([128, CH], F32, tag="up", name=f"uo{ch}")
            for k in range(NMQ):
                nc.tensor.matmul(psO[:], wuqro_t[:, k, :], cq[:, k, :],
                                 start=(k == 0), stop=(k == NMQ - 1))
            osc = rope3.tile([128, CH], F32, tag="osc", bufs=1, name=f"osc{ch}")
            nc.scalar.activation(osc[:], psO[:], AF.Copy)
            t1 = rope3.tile([128, CH], F32, tag="t1", bufs=1, name=f"t1{ch}")
            nc.vector.tensor_mul(t1[:], esc[:], cos_t[:])
            t2 = rope3.tile([128, CH], F32, tag="t2", bufs=1, name=f"t2{ch}")
            nc.vector.tensor_mul(t2[:], osc[:], sin_t[:])
            o1 = rope3.tile([128, CH], F32R, tag="o1", name=f"o1{ch}")
            nc.vector.tensor_sub(o1[:], t1[:], t2[:])
            t3 = rope3.tile([128, CH], F32, tag="t1", bufs=1, name=f"t3{ch}")
            nc.vector.tensor_mul(t3[:], esc[:], sin_t[:])
            t4 = rope3.tile([128, CH], F32, tag="t2", bufs=1, name=f"t4{ch}")
            nc.vector.tensor_mul(t4[:], osc[:], cos_t[:])
            o2 = rope3.tile([128, CH], F32R, tag="o2", name=f"o2{ch}")
            nc.vector.tensor_add(o2[:], t3[:], t4[:])
            for h in range(HL):
                p, off = h // 2, (h % 2) * 64
                nc.sync.dma_start(qrope[p][off:off + 32, sl], o1[ts(h, 32), :])
                nc.sync.dma_start(qrope[p][off + 32:off + 64, sl],
                                    o2[ts(h, 32), :])

    # ============ P4: block-causal attention + W_O ============
    with tc.tile_pool(name="wo", bufs=1) as wop, \
         tc.tile_pool(name="maskP", bufs=1) as maskpl, \
         tc.tile_pool(name="pu", bufs=2) as pup, \
         tc.tile_pool(name="pT", bufs=2) as pTp, \
         tc.tile_pool(name="attP", bufs=1) as attp, \
         tc.tile_pool(name="osb", bufs=2) as osb, \
         tc.tile_pool(name="stats", bufs=4) as stats, \
         tc.tile_pool(name="psS", bufs=6, space="PSUM") as psS, \
         tc.tile_pool(name="psAV", bufs=1, space="PSUM") as psAV, \
         tc.tile_pool(name="psWO", bufs=1, space="PSUM") as psWO:
        wo_t = wop.tile([128, HL, D], BF16, tag="wo")
        nc.sync.dma_start(wo_t[:], wo4.rearrange("(ht p) m -> p ht m", p=128))
        mts = []
        for qt in range(4):
            mt = maskpl.tile([128, 512], BF16, tag=f"mask{qt}", name=f"mk{qt}")
            nc.sync.dma_start(mt[:], maskp[ts(qt, 128), :])
            mts.append(mt)
        for qb in range(4):
            nv = qb + 1              # valid 512-wide key chunks
            vw = nv * 512            # valid key width
            att = [None] * HL
            pTs = [None] * HL

            def do_av(h):
                pav = psAV.tile([128, 512], F32, tag="av", name=f"av{qb}_{h}")
                for kt in range(4 * nv):
                    nc.tensor.matmul(pav[:], v_all[:, kt, ts(h, 128)],
                                     pTs[h][:, kt, :],
                                     start=(kt == 0), stop=(kt == 4 * nv - 1))
                at = attp.tile([128, 512], BF16, tag=f"att{h}", name=f"at{qb}_{h}")
                nc.scalar.activation(at[:], pav[:], AF.Copy)
                att[h] = at

            for h in range(HL):
                pT_t = pTp.tile([128, NQT, 512], BF16, tag="pT", name=f"pT{qb}_{h}")
                pTs[h] = pT_t
                for qt in range(4):
                    qsl = slice((qb * 4 + qt) * 128, (qb * 4 + qt + 1) * 128)
                    phc = [psS.tile([128, 512], F32, tag="qk",
                                    name=f"qk{qb}_{h}_{qt}_{c}")
                           for c in range(nv)]
                    off = (h % 2) * 64
                    for c in range(nv):
                        pp = phc[c][:]
                        ksl = slice(c * 512, (c + 1) * 512)
                        nc.tensor.matmul(pp, qT[h][:, qsl], kT[h][:, ksl],
                                         start=True, stop=False)
                        nc.tensor.matmul(pp, qrope[h // 2][off:off + 64, qsl],
                                         krope2[off:off + 64, ksl],
                                         start=False, stop=(c != qb))
                        if c == qb:
                            nc.tensor.matmul(pp, ident_t[:], mts[qt][:],
                                             start=False, stop=True)
                    mxs = []
                    for c in range(nv):
                        mx = stats.tile([128, 1], F32, tag=f"mx{c}",
                                        name=f"mx{qb}{h}{qt}_{c}")
                        nc.vector.reduce_max(mx[:], phc[c][:], axis=AX.X)
                        mxs.append(mx)
                    mxc = mxs[0]
                    for c in range(1, nv):
                        t = stats.tile([128, 1], F32, tag=f"cmb{c}",
                                       name=f"cmb{qb}{h}{qt}_{c}")
                        nc.vector.tensor_max(t[:], mxc[:], mxs[c][:])
                        mxc = t
                    negm = stats.tile([128, 1], F32, tag="negm", name=f"ng{qb}{h}{qt}")
                    nc.vector.tensor_scalar_mul(negm[:], mxc[:], -SCALE)
                    pu = pup.tile([128, S], BF16, tag="pu", name=f"pu{qb}{h}{qt}")
                    las = []
                    for c in range(nv):
                        la = stats.tile([128, 1], F32, tag=f"la{c}",
                                        name=f"la{qb}{h}{qt}_{c}")
                        nc.scalar.activation(pu[:, ts(c, 512)], phc[c][:], AF.Exp,
                                             bias=negm[:], scale=SCALE,
                                             accum_out=la[:])
                        las.append(la)
                    lt = las[0]
                    for c in range(1, nv):
                        t2 = stats.tile([128, 1], F32, tag=f"lts{c}",
                                        name=f"lts{qb}{h}{qt}_{c}")
                        nc.vector.tensor_add(t2[:], lt[:], las[c][:])
                        lt = t2
                    rl = stats.tile([128, 1], F32, tag="rl", name=f"rl{qb}{h}{qt}")
                    nc.vector.reciprocal(rl[:], lt[:])
                    nc.vector.tensor_scalar_mul(pu[:, 0:vw], pu[:, 0:vw], rl[:])
                    nc.sync.dma_start(pT_t[:, 0:4 * nv, ts(qt, 128)], pu[:, 0:vw],
                                      transpose=True)
                if h > 0:
                    do_av(h - 1)
            do_av(HL - 1)
            for qt in range(4):
                qrow = (qb * 4 + qt) * 128
                ot = osb.tile([128, D], F32, tag="ot", name=f"ot{qb}{qt}")
                for dch in range(4):
                    pw = psWO.tile([128, 512], F32, tag="wops", name=f"wo{qb}{qt}{dch}")
                    for h in range(HL):
                        nc.tensor.matmul(pw[:], att[h][:, ts(qt, 128)],
                                         wo_t[:, h, ts(dch, 512)],
                                         start=(h == 0), stop=(h == HL - 1))
                    nc.scalar.activation(ot[:, ts(dch, 512)], pw[:], AF.Copy)
                nc.gpsimd.dma_start(outp[qrow:qrow + 128, :], ot[:])

    qside_p.__exit__(None, None, None)
    kside_p.__exit__(None, None, None)
    dram_p.__exit__(None, None, None)
    const_p.__exit__(None, None, None)


def _shard(inputs):
    x = np.asarray(inputs["x"], np.float32)
    mask = np.asarray(inputs["mask"], np.float32)[0, 0]
    pos_cos = np.asarray(inputs["pos_cos"], np.float32)
    pos_sin = np.asarray(inputs["pos_sin"], np.float32)
    W_DQ = np.asarray(inputs["W_DQ"], np.float32)
    W_UQ = np.asarray(inputs["W_UQ"], np.float32)
    W_UQR = np.asarray(inputs["W_UQR"], np.float32)
    W_DKV = np.asarray(inputs["W_DKV"], np.float32)
    W_UK = np.asarray(inputs["W_UK"], np.float32)
    W_UV = np.asarray(inputs["W_UV"], np.float32)
    W_DKR = np.asarray(inputs["W_DKR"], np.float32)
    W_O = np.asarray(inputs["W_O"], np.float32)
    qw = np.asarray(inputs["q_norm_w"], np.float32)
    kvw = np.asarray(inputs["kv_norm_w"], np.float32)

    maskp = (mask[0:512, 0:512] / SCALE).astype(BF)
    cos4 = np.tile(np.ascontiguousarray(pos_cos.T), (4, 1)).astype(np.float32)
    sin4 = np.tile(np.ascontiguousarray(pos_sin.T), (4, 1)).astype(np.float32)
    wdkr = np.ascontiguousarray(
        np.concatenate([W_DKR[:, 0::2], W_DKR[:, 1::2]], axis=1))
    wuq_n = W_UQ * qw[:, None]
    wuqr_n = (W_UQR * qw[:, None]).reshape(DCQ, H, ROPE)
    wuk_n = W_UK * kvw[:, None]
    wuv_n = W_UV * kvw[:, None]
    ident = np.eye(128, dtype=np.float32).astype(BF)
    ones_r = np.ones((1, 128), np.float32)
    ones_c = np.ones((128, 1), np.float32)

    in_maps = []
    for c in range(NCORES):
        b, g = divmod(c, 4)
        hs = slice(g * HL * NOPE, (g + 1) * HL * NOPE)
        heads = list(range(g * HL, (g + 1) * HL))
        wuqre = np.concatenate([wuqr_n[:, h, 0::2] for h in heads], axis=1)
        wuqro = np.concatenate([wuqr_n[:, h, 1::2] for h in heads], axis=1)
        in_maps.append({
            "xT": np.ascontiguousarray(x[b].T),
            "maskp": maskp,
            "cos4": cos4,
            "sin4": sin4,
            "wdq": W_DQ,
            "wdkv": W_DKV,
            "wdkr": wdkr,
            "wuq": np.ascontiguousarray(wuq_n[:, hs]),
            "wuqre": np.ascontiguousarray(wuqre),
            "wuqro": np.ascontiguousarray(wuqro),
            "wuk": np.ascontiguousarray(wuk_n[:, hs]),
            "wuv": np.ascontiguousarray(wuv_n[:, hs]),
            "wo4": np.ascontiguousarray(W_O[hs, :]).astype(BF),
            "ident": ident,
            "ones_r": ones_r,
            "ones_c": ones_c,
        })
    return in_maps


def kernel(**inputs):
    from concourse.bass_utils import run_bass_kernel_spmd

    if "nc" not in _BUILD_CACHE:
        _BUILD_CACHE["nc"] = build_nc()
    nc = _BUILD_CACHE["nc"]
    in_maps = _shard(inputs)
    res = run_bass_kernel_spmd(nc, in_maps, core_ids=list(range(NCORES)))
    out = np.zeros((B, S, D), np.float32)
    for c in range(NCORES):
        out[c // 4] += np.asarray(res.results[c]["outp"], np.float32)
    return out


# revision 12
# speedup vs baseline: 1.0521x; 1.0486x over previous
"""MLA (DeepSeek-style) attention layer on 8 Trainium2 NeuronCores.

Sharding: core c -> batch b = c//4, head group g = c%4 (4 of 16 heads).
Each core computes a partial output (its heads' contribution through its
W_O row-slice); the host sums the 4 partials per batch.

v2: four phases, causal attention.
  P1: c_Q down-proj (rms-scaled in-phase), spilled to DRAM.
  P2: c_KV down-proj fused with k/v up-projection + k-rope (no spill).
  P3: c_Q reload -> q up-projection + q-rope.
  P4: block-causal attention (diag-chunk masks only) + row-parallel W_O.
Weights are loaded once each in single large DMAs; x is streamed twice
(P1/P2) in one DMA per 256-token chunk. DMA issue is spread over the
Pool (SWDGE) and SP/ACT (HWDGE) queues to avoid sequencer serialization.
The q/k path stays fp32r end-to-end for logit accuracy; P/V/W_O run bf16.
"""
import sys

for _p in ("/opt/trn_rl_repo", "/root/.axon_site/_ro/trn_rl_repo"):
    if _p not in sys.path:
        sys.path.append(_p)

import numpy as np
import ml_dtypes

B, S, D = 2, 2048, 2048
H, NOPE, ROPE, VD = 16, 128, 64, 128
DCQ, DCKV = 1536, 512
EPS = 1e-6
SCALE = float(np.sqrt(NOPE + ROPE))
HL = 4           # local heads per core
NCORES = 8
NQT = S // 128   # 16
NKC = S // 512   # 4
NKD = D // 128   # 16
NMQ = DCQ // 128  # 12
NMKV = DCKV // 128  # 4
CH = 256         # token chunk for P1-P3
NCH = S // CH    # 8
BF = ml_dtypes.bfloat16

_BUILD_CACHE = {}


def build_nc():
    import concourse.tile as tile
    import concourse.mybir as mybir
    from concourse import bacc

    F32 = mybir.dt.float32
    F32R = mybir.dt.float32r
    BF16 = mybir.dt.bfloat16

    nc = bacc.Bacc(num_devices=NCORES)

    T = {}
    T["xT"] = nc.dram_tensor("xT", [D, S], F32R, kind="ExternalInput")
    T["maskp"] = nc.dram_tensor("maskp", [512, 512], BF16, kind="ExternalInput")
    T["cos4"] = nc.dram_tensor("cos4", [128, S], F32, kind="ExternalInput")
    T["sin4"] = nc.dram_tensor("sin4", [128, S], F32, kind="ExternalInput")
    T["wdq"] = nc.dram_tensor("wdq", [D, DCQ], F32R, kind="ExternalInput")
    T["wdkv"] = nc.dram_tensor("wdkv", [D, DCKV], F32R, kind="ExternalInput")
    T["wdkr"] = nc.dram_tensor("wdkr", [D, 64], F32R, kind="ExternalInput")
    T["wuq"] = nc.dram_tensor("wuq", [DCQ, HL * NOPE], F32R, kind="ExternalInput")
    T["wuqre"] = nc.dram_tensor("wuqre", [DCQ, HL * 32], F32R, kind="ExternalInput")
    T["wuqro"] = nc.dram_tensor("wuqro", [DCQ, HL * 32], F32R, kind="ExternalInput")
    T["wuk"] = nc.dram_tensor("wuk", [DCKV, HL * NOPE], F32R, kind="ExternalInput")
    T["wuv"] = nc.dram_tensor("wuv", [DCKV, HL * VD], F32R, kind="ExternalInput")
    T["wo4"] = nc.dram_tensor("wo4", [HL * VD, D], BF16, kind="ExternalInput")
    T["ident"] = nc.dram_tensor("ident", [128, 128], BF16, kind="ExternalInput")
    T["ones_r"] = nc.dram_tensor("ones_r", [1, 128], F32R, kind="ExternalInput")
    T["ones_c"] = nc.dram_tensor("ones_c", [128, 1], F32R, kind="ExternalInput")
    T["outp"] = nc.dram_tensor("outp", [S, D], F32, kind="ExternalOutput")

    with tile.TileContext(nc) as tc:
        _emit(nc, tc, T)
    nc.compile()
    return nc


def _emit(nc, tc, T):
    import concourse.bass as bass
    import concourse.mybir as mybir

    F32 = mybir.dt.float32
    F32R = mybir.dt.float32r
    BF16 = mybir.dt.bfloat16
    AF = mybir.ActivationFunctionType
    AX = mybir.AxisListType
    ts = bass.ts

    xT, maskp, cos4, sin4 = T["xT"], T["maskp"], T["cos4"], T["sin4"]
    wdq, wdkv, wdkr = T["wdq"], T["wdkv"], T["wdkr"]
    wuq, wuqre, wuqro, wuk, wuv, wo4 = (
        T["wuq"], T["wuqre"], T["wuqro"], T["wuk"], T["wuv"], T["wo4"])
    ident, ones_r, ones_c, outp = T["ident"], T["ones_r"], T["ones_c"], T["outp"]

    xTr = xT.rearrange("(kt p) s -> p kt s", p=128)

    # --- persistent-scope pools, opened in lifetime (LIFO) order ---
    const_p = tc.tile_pool(name="constp", bufs=1)
    const = const_p.__enter__()
    onesr_t = const.tile([1, 128], F32R, tag="onesr")
    nc.sync.dma_start(onesr_t[:], ones_r[:])
    onesc_t = const.tile([128, 1], F32R, tag="onesc")
    nc.sync.dma_start(onesc_t[:], ones_c[:])
    ident_t = const.tile([128, 128], BF16, tag="ident")
    nc.sync.dma_start(ident_t[:], ident[:])
    epst = const.tile([1, 1], F32, tag="epst")
    nc.gpsimd.memset(epst[:], EPS)

    dram_p = tc.tile_pool(name="dram", bufs=1, space="DRAM")
    dram = dram_p.__enter__()
    cqd = dram.tile([128, NMQ, S], F32R, tag="cqd")

    # ============ P1: c_Q down-projection, rms-scaled, spilled ============
    with tc.tile_pool(name="wP1", bufs=1) as wp1, \
         tc.tile_pool(name="xP1", bufs=2) as xp1, \
         tc.tile_pool(name="evP1", bufs=2) as evp1, \
         tc.tile_pool(name="sqP1", bufs=2) as sqp1, \
         tc.tile_pool(name="invP1", bufs=2) as invp1, \
         tc.tile_pool(name="psP1", bufs=3, space="PSUM") as psp1, \
         tc.tile_pool(name="psS1", bufs=2, space="PSUM") as pss1:
        wdq_t = wp1.tile([128, NKD, DCQ], F32R, tag="wdq")
        wdqr = wdq.rearrange("(kt p) m -> p kt m", p=128)
        for k in range(NKD):
            nc.sync.dma_start(wdq_t[:, k, :], wdqr[:, k, :])
        for ch in range(NCH):
            sl = slice(ch * CH, (ch + 1) * CH)
            xq = xp1.tile([128, NKD, CH], F32R, tag="xq", name=f"xq{ch}")
            nc.gpsimd.dma_start(xq[:], xTr[:, :, sl])
            ev = evp1.tile([128, NMQ, CH], F32R, tag="ev", name=f"ev{ch}")
            sum_ps = pss1.tile([1, CH], F32, tag="sum", name=f"sum{ch}")
            for m in range(NMQ):
                ps = psp1.tile([128, CH], F32, tag="dp", name=f"dp{ch}_{m}")
                for k in range(NKD):
                    nc.tensor.matmul(ps[:], wdq_t[:, k, ts(m, 128)], xq[:, k, :],
                                     start=(k == 0), stop=(k == NKD - 1))
                nc.scalar.activation(ev[:, m, :], ps[:], AF.Copy)
                sq = sqp1.tile([128, CH], F32R, tag="sq", name=f"sq{ch}_{m}")
                nc.scalar.activation(sq[:], ps[:], AF.Square)
                nc.tensor.matmul(sum_ps[:], onesc_t[:], sq[:],
                                 start=(m == 0), stop=(m == NMQ - 1))
            rms = invp1.tile([1, CH], F32, tag="rms", name=f"rms{ch}")
            nc.scalar.activation(rms[:], sum_ps[:], AF.Sqrt, bias=epst[:],
                                 scale=1.0 / DCQ)
            inv = invp1.tile([1, CH], F32R, tag="inv", name=f"inv{ch}")
            with nc.allow_low_precision(reason="f32r shares f32 bits"):
                nc.vector.reciprocal(inv[:], rms[:])
            psb = pss1.tile([128, CH], F32, tag="bc", name=f"bc{ch}")
            nc.tensor.matmul(psb[:], onesr_t[:], inv[:], start=True, stop=True)
            invbc = invp1.tile([128, CH], F32R, tag="invbc", name=f"invbc{ch}")
            nc.scalar.activation(invbc[:], psb[:], AF.Copy)
            for m in range(NMQ):
                nc.vector.tensor_mul(ev[:, m, :], ev[:, m, :], invbc[:])
            nc.sync.dma_start(cqd[:, :, sl], ev[:])

    # ============ P2: c_KV down-proj fused with k/v up-proj + k-rope ======
    kside_p = tc.tile_pool(name="kside", bufs=1)
    kside = kside_p.__enter__()
    kT = [kside.tile([128, S], F32R, tag=f"kT{h}", name=f"kT{h}") for h in range(HL)]
    krope2 = kside.tile([128, S], F32R, tag="krope2")
    v_all = kside.tile([128, NQT, HL * VD], BF16, tag="v_all")

    with tc.tile_pool(name="wP2", bufs=1) as wp2, \
         tc.tile_pool(name="xP2", bufs=2) as xp2, \
         tc.tile_pool(name="evP2", bufs=2) as evp2, \
         tc.tile_pool(name="sqP2", bufs=2) as sqp2, \
         tc.tile_pool(name="invP2", bufs=2) as invp2, \
         tc.tile_pool(name="ropeP2", bufs=2) as rope2, \
         tc.tile_pool(name="psP2", bufs=2, space="PSUM") as psp2, \
         tc.tile_pool(name="psS2", bufs=1, space="PSUM") as pss2, \
         tc.tile_pool(name="psU2", bufs=2, space="PSUM") as psu2:
        wdkv_t = wp2.tile([128, NKD, DCKV], F32R, tag="wdkv")
        wdkvr = wdkv.rearrange("(kt p) m -> p kt m", p=128)
        for k in range(NKD):
            nc.sync.dma_start(wdkv_t[:, k, :], wdkvr[:, k, :])
        wdkr_t = wp2.tile([128, NKD, 64], F32R, tag="wdkr")
        nc.sync.dma_start(wdkr_t[:], wdkr.rearrange("(kt p) m -> p kt m", p=128))
        wuk_t = wp2.tile([128, NMKV, HL * NOPE], F32R, tag="wuk")
        wukr = wuk.rearrange("(kt p) m -> p kt m", p=128)
        wuv_t = wp2.tile([128, NMKV, HL * VD], F32R, tag="wuv")
        wuvr = wuv.rearrange("(kt p) m -> p kt m", p=128)
        for k in range(NMKV):
            nc.sync.dma_start(wuk_t[:, k, :], wukr[:, k, :])
            nc.sync.dma_start(wuv_t[:, k, :], wuvr[:, k, :])
        for ch in range(NCH):
            sl = slice(ch * CH, (ch + 1) * CH)
            xq = xp2.tile([128, NKD, CH], F32R, tag="xq", name=f"x2{ch}")
            nc.gpsimd.dma_start(xq[:], xTr[:, :, sl])
            ev = evp2.tile([128, NMKV, CH], F32R, tag="ev", name=f"e2{ch}")
            sum_ps = pss2.tile([1, CH], F32, tag="sum", name=f"s2{ch}")
            for m in range(NMKV):
                ps = psp2.tile([128, CH], F32, tag="dp", name=f"d2{ch}_{m}")
                for k in range(NKD):
                    nc.tensor.matmul(ps[:], wdkv_t[:, k, ts(m, 128)], xq[:, k, :],
                                     start=(k == 0), stop=(k == NKD - 1))
                nc.scalar.activation(ev[:, m, :], ps[:], AF.Copy)
                sq = sqp2.tile([128, CH], F32R, tag="sq", name=f"q2{ch}_{m}")
                nc.scalar.activation(sq[:], ps[:], AF.Square)
                nc.tensor.matmul(sum_ps[:], onesc_t[:], sq[:],
                                 start=(m == 0), stop=(m == NMKV - 1))
            # k_R = x @ W_DKR (no rms norm), rope'd below
            pskr = psp2.tile([128, CH], F32, tag="dp", name=f"kr{ch}")
            for k in range(NKD):
                nc.tensor.matmul(pskr[:64, :], wdkr_t[:, k, :], xq[:, k, :],
                                 start=(k == 0), stop=(k == NKD - 1))
            kre = rope2.tile([32, CH], F32, tag="kre", name=f"kre{ch}")
            nc.scalar.activation(kre[:], pskr[0:32, :], AF.Copy)
            kro = rope2.tile([32, CH], F32, tag="kro", name=f"kro{ch}")
            nc.scalar.activation(kro[:], pskr[32:64, :], AF.Copy)
            rms = invp2.tile([1, CH], F32, tag="rms", name=f"r2{ch}")
            nc.scalar.activation(rms[:], sum_ps[:], AF.Sqrt, bias=epst[:],
                                 scale=1.0 / DCKV)
            inv = invp2.tile([1, CH], F32R, tag="inv", name=f"i2{ch}")
            with nc.allow_low_precision(reason="f32r shares f32 bits"):
                nc.vector.reciprocal(inv[:], rms[:])
            psb = pss2.tile([128, CH], F32, tag="bc", name=f"b2{ch}")
            nc.tensor.matmul(psb[:], onesr_t[:], inv[:], start=True, stop=True)
            invbc = invp2.tile([128, CH], F32R, tag="invbc", name=f"ib2{ch}")
            nc.scalar.activation(invbc[:], psb[:], AF.Copy)
            for m in range(NMKV):
                nc.vector.tensor_mul(ev[:, m, :], ev[:, m, :], invbc[:])
            # k-rope for this chunk
            cs_a = rope2.tile([32, CH], F32, tag="cs_a", name=f"cs{ch}")
            nc.gpsimd.dma_start(cs_a[:], cos4[0:32, sl])
            sn_a = rope2.tile([32, CH], F32, tag="sn_a", name=f"sn{ch}")
            nc.gpsimd.dma_start(sn_a[:], sin4[0:32, sl])
            t1k = rope2.tile([32, CH], F32, tag="t1k", bufs=1, name=f"t1k{ch}")
            nc.vector.tensor_mul(t1k[:], kre[:], cs_a[:])
            t2k = rope2.tile([32, CH], F32, tag="t2k", bufs=1, name=f"t2k{ch}")
            nc.vector.tensor_mul(t2k[:], kro[:], sn_a[:])
            ko1 = rope2.tile([32, CH], F32R, tag="ko1", name=f"ko1{ch}")
            nc.vector.tensor_sub(ko1[:], t1k[:], t2k[:])
            t3k = rope2.tile([32, CH], F32, tag="t3k", bufs=1, name=f"t3k{ch}")
            nc.vector.tensor_mul(t3k[:], kre[:], sn_a[:])
            t4k = rope2.tile([32, CH], F32, tag="t4k", bufs=1, name=f"t4k{ch}")
            nc.vector.tensor_mul(t4k[:], kro[:], cs_a[:])
            ko2 = rope2.tile([32, CH], F32R, tag="ko2", name=f"ko2{ch}")
            nc.vector.tensor_add(ko2[:], t3k[:], t4k[:])
            for rep in range(2):
                nc.sync.dma_start(krope2[ts(rep * 2, 32), sl], ko1[:])
                nc.sync.dma_start(krope2[ts(rep * 2 + 1, 32), sl], ko2[:])
            # k up-projection
            for h in range(HL):
                ps = psu2.tile([128, CH], F32, tag="upk", name=f"uk{ch}_{h}")
                for k in range(NMKV):
                    nc.tensor.matmul(ps[:], wuk_t[:, k, ts(h, 128)], ev[:, k, :],
                                     start=(k == 0), stop=(k == NMKV - 1))
                nc.scalar.activation(kT[h][:, sl], ps[:], AF.Copy)
            # v up-projection (token-major output)
            for vm in range(CH // 128):
                m = ch * (CH // 128) + vm
                ps = psu2.tile([128, HL * VD], F32, tag="upv", name=f"uv{ch}_{vm}")
                for k in range(NMKV):
                    nc.tensor.matmul(ps[:], ev[:, k, ts(vm, 128)], wuv_t[:, k, :],
                                     start=(k == 0), stop=(k == NMKV - 1))
                nc.scalar.activation(v_all[:, m, :], ps[:], AF.Copy)

    # ============ P3: q up-projection + q-rope ============
    qside_p = tc.tile_pool(name="qside", bufs=1)
    qside = qside_p.__enter__()
    qT = [qside.tile([128, S], F32R, tag=f"qT{h}", name=f"qT{h}") for h in range(HL)]
    qrope = [qside.tile([128, S], F32R, tag=f"qrope{p}", name=f"qrope{p}")
             for p in range(2)]

    with tc.tile_pool(name="wP3", bufs=1) as wp3, \
         tc.tile_pool(name="cqP3", bufs=2) as cqp3, \
         tc.tile_pool(name="csP3", bufs=2) as csp3, \
         tc.tile_pool(name="ropeP3", bufs=2) as rope3, \
         tc.tile_pool(name="psP3", bufs=3, space="PSUM") as psp3:
        wuq_t = wp3.tile([128, NMQ, HL * NOPE], F32R, tag="wuq")
        wuqr_ = wuq.rearrange("(kt p) m -> p kt m", p=128)
        wuqre_t = wp3.tile([128, NMQ, HL * 32], F32R, tag="wuqre")
        wuqrer = wuqre.rearrange("(kt p) m -> p kt m", p=128)
        wuqro_t = wp3.tile([128, NMQ, HL * 32], F32R, tag="wuqro")
        wuqror = wuqro.rearrange("(kt p) m -> p kt m", p=128)
        for k in range(NMQ):
            nc.sync.dma_start(wuq_t[:, k, :], wuqr_[:, k, :])
            nc.sync.dma_start(wuqre_t[:, k, :], wuqrer[:, k, :])
            nc.sync.dma_start(wuqro_t[:, k, :], wuqror[:, k, :])
        for ch in range(NCH):
            sl = slice(ch * CH, (ch + 1) * CH)
            cq = cqp3.tile([128, NMQ, CH], F32R, tag="cq", name=f"cq{ch}")
            nc.gpsimd.dma_start(cq[:], cqd[:, :, sl])
            cos_t = csp3.tile([128, CH], F32, tag="cos", name=f"cos{ch}")
            nc.gpsimd.dma_start(cos_t[:], cos4[:, sl])
            sin_t = csp3.tile([128, CH], F32, tag="sin", name=f"sin{ch}")
            nc.gpsimd.dma_start(sin_t[:], sin4[:, sl])
            for h in range(HL):
                ps = psp3.tile([128, CH], F32, tag="up", name=f"uq{ch}_{h}")
                for k in range(NMQ):
                    nc.tensor.matmul(ps[:], wuq_t[:, k, ts(h, 128)], cq[:, k, :],
                                     start=(k == 0), stop=(k == NMQ - 1))
                nc.scalar.activation(qT[h][:, sl], ps[:], AF.Copy)
            psE = psp3.tile([128, CH], F32, tag="up", name=f"ue{ch}")
            for k in range(NMQ):
                nc.tensor.matmul(psE[:], wuqre_t[:, k, :], cq[:, k, :],
                                 start=(k == 0), stop=(k == NMQ - 1))
            esc = rope3.tile([128, CH], F32, tag="esc", bufs=1, name=f"esc{ch}")
            nc.scalar.activation(esc[:], psE[:], AF.Copy)
            psO = psp3.tile([128, CH], F32, tag="up", name=f"uo{ch}")
            for k in range(NMQ):
                nc.tensor.matmul(psO[:], wuqro_t[:, k, :], cq[:, k, :],
                                 start=(k == 0), stop=(k == NMQ - 1))
            osc = rope3.tile([128, CH], F32, tag="osc", bufs=1, name=f"osc{ch}")
            nc.scalar.activation(osc[:], psO[:], AF.Copy)
            t1 = rope3.tile([128, CH], F32, tag="t1", bufs=1, name=f"t1{ch}")
            nc.vector.tensor_mul(t1[:], esc[:], cos_t[:])
            t2 = rope3.tile([128, CH], F32, tag="t2", bufs=1, name=f"t2{ch}")
            nc.vector.tensor_mul(t2[:], osc[:], sin_t[:])
            o1 = rope3.tile([128, CH], F32R, tag="o1", name=f"o1{ch}")
            nc.vector.tensor_sub(o1[:], t1[:], t2[:])
            t3 = rope3.tile([128, CH], F32, tag="t1", bufs=1, name=f"t3{ch}")
            nc.vector.tensor_mul(t3[:], esc[:], sin_t[:])
            t4 = rope3.tile([128, CH], F32, tag="t2", bufs=1, name=f"t4{ch}")
            nc.vector.tensor_mul(t4[:], osc[:], cos_t[:])
            o2 = rope3.tile([128, CH], F32R, tag="o2", name=f"o2{ch}")
            nc.vector.tensor_add(o2[:], t3[:], t4[:])
            for h in range(HL):
                p, off = h // 2, (h % 2) * 64
                nc.sync.dma_start(qrope[p][off:off + 32, sl], o1[ts(h, 32), :])
                nc.sync.dma_start(qrope[p][off + 32:off + 64, sl],
                                    o2[ts(h, 32), :])

    # ============ P4: block-causal attention + W_O ============
    with tc.tile_pool(name="wo", bufs=1) as wop, \
         tc.tile_pool(name="maskP", bufs=1) as maskpl, \
         tc.tile_pool(name="pu", bufs=4) as pup, \
         tc.tile_pool(name="pT", bufs=2) as pTp, \
         tc.tile_pool(name="attP", bufs=2) as attp, \
         tc.tile_pool(name="osb", bufs=2) as osb, \
         tc.tile_pool(name="stats", bufs=4) as stats, \
         tc.tile_pool(name="psS", bufs=6, space="PSUM") as psS, \
         tc.tile_pool(name="psAV", bufs=1, space="PSUM") as psAV, \
         tc.tile_pool(name="psWO", bufs=1, space="PSUM") as psWO:
        wo_t = wop.tile([128, HL, D], BF16, tag="wo")
        nc.sync.dma_start(wo_t[:], wo4.rearrange("(ht p) m -> p ht m", p=128))
        mts = []
        for qt in range(4):
            mt = maskpl.tile([128, 512], BF16, tag=f"mask{qt}", name=f"mk{qt}")
            nc.sync.dma_start(mt[:], maskp[ts(qt, 128), :])
            mts.append(mt)
        for qb in range(4):
            nv = qb + 1              # valid 512-wide key chunks
            vw = nv * 512            # valid key width
            att = [None] * HL
            pTs = [None] * HL

            def do_av(h):
                pav = psAV.tile([128, 512], F32, tag="av", name=f"av{qb}_{h}")
                for kt in range(4 * nv):
                    nc.tensor.matmul(pav[:], v_all[:, kt, ts(h, 128)],
                                     pTs[h][:, kt, :],
                                     start=(kt == 0), stop=(kt == 4 * nv - 1))
                at = attp.tile([128, 512], BF16, tag=f"att{h}", name=f"at{qb}_{h}")
                nc.scalar.activation(at[:], pav[:], AF.Copy)
                att[h] = at

            for h in range(HL):
                pT_t = pTp.tile([128, NQT, 512], BF16, tag="pT", name=f"pT{qb}_{h}")
                pTs[h] = pT_t
                for qt in range(4):
                    qsl = slice((qb * 4 + qt) * 128, (qb * 4 + qt + 1) * 128)
                    phc = [psS.tile([128, 512], F32, tag="qk",
                                    name=f"qk{qb}_{h}_{qt}_{c}")
                           for c in range(nv)]
                    off = (h % 2) * 64
                    for c in range(nv):
                        pp = phc[c][:]
                        ksl = slice(c * 512, (c + 1) * 512)
                        nc.tensor.matmul(pp, qT[h][:, qsl], kT[h][:, ksl],
                                         start=True, stop=False)
                        nc.tensor.matmul(pp, qrope[h // 2][off:off + 64, qsl],
                                         krope2[off:off + 64, ksl],
                                         start=False, stop=(c != qb))
                        if c == qb:
                            nc.tensor.matmul(pp, ident_t[:], mts[qt][:],
                                             start=False, stop=True)
                    mxs = []
                    for c in range(nv):
                        mx = stats.tile([128, 1], F32, tag=f"mx{c}",
                                        name=f"mx{qb}{h}{qt}_{c}")
                        nc.vector.reduce_max(mx[:], phc[c][:], axis=AX.X)
                        mxs.append(mx)
                    mxc = mxs[0]
                    for c in range(1, nv):
                        t = stats.tile([128, 1], F32, tag=f"cmb{c}",
                                       name=f"cmb{qb}{h}{qt}_{c}")
                        nc.vector.tensor_max(t[:], mxc[:], mxs[c][:])
                        mxc = t
                    negm = stats.tile([128, 1], F32, tag="negm", name=f"ng{qb}{h}{qt}")
                    nc.vector.tensor_scalar_mul(negm[:], mxc[:], -SCALE)
                    pu = pup.tile([128, S], BF16, tag="pu", name=f"pu{qb}{h}{qt}")
                    las = []
                    for c in range(nv):
                        la = stats.tile([128, 1], F32, tag=f"la{c}",
                                        name=f"la{qb}{h}{qt}_{c}")
                        nc.scalar.activation(pu[:, ts(c, 512)], phc[c][:], AF.Exp,
                                             bias=negm[:], scale=SCALE,
                                             accum_out=la[:])
                        las.append(la)
                    lt = las[0]
                    for c in range(1, nv):
                        t2 = stats.tile([128, 1], F32, tag=f"lts{c}",
                                        name=f"lts{qb}{h}{qt}_{c}")
                        nc.vector.tensor_add(t2[:], lt[:], las[c][:])
                        lt = t2
                    rl = stats.tile([128, 1], F32, tag="rl", name=f"rl{qb}{h}{qt}")
                    nc.vector.reciprocal(rl[:], lt[:])
                    nc.vector.tensor_scalar_mul(pu[:, 0:vw], pu[:, 0:vw], rl[:])
                    nc.sync.dma_start(pT_t[:, 0:4 * nv, ts(qt, 128)], pu[:, 0:vw],
                                      transpose=True)
                if h > 0:
                    do_av(h - 1)
            do_av(HL - 1)
            for qt in range(4):
                qrow = (qb * 4 + qt) * 128
                ot = osb.tile([128, D], F32, tag="ot", name=f"ot{qb}{qt}")
                for dch in range(4):
                    pw = psWO.tile([128, 512], F32, tag="wops", name=f"wo{qb}{qt}{dch}")
                    for h in range(HL):
                        nc.tensor.matmul(pw[:], att[h][:, ts(qt, 128)],
                                         wo_t[:, h, ts(dch, 512)],
                                         start=(h == 0), stop=(h == HL - 1))
                    nc.scalar.activation(ot[:, ts(dch, 512)], pw[:], AF.Copy)
                nc.gpsimd.dma_start(outp[qrow:qrow + 128, :], ot[:])

    qside_p.__exit__(None, None, None)
    kside_p.__exit__(None, None, None)
    dram_p.__exit__(None, None, None)
    const_p.__exit__(None, None, None)


def _shard(inputs):
    x = np.asarray(inputs["x"], np.float32)
    mask = np.asarray(inputs["mask"], np.float32)[0, 0]
    pos_cos = np.asarray(inputs["pos_cos"], np.float32)
    pos_sin = np.asarray(inputs["pos_sin"], np.float32)
    W_DQ = np.asarray(inputs["W_DQ"], np.float32)
    W_UQ = np.asarray(inputs["W_UQ"], np.float32)
    W_UQR = np.asarray(inputs["W_UQR"], np.float32)
    W_DKV = np.asarray(inputs["W_DKV"], np.float32)
    W_UK = np.asarray(inputs["W_UK"], np.float32)
    W_UV = np.asarray(inputs["W_UV"], np.float32)
    W_DKR = np.asarray(inputs["W_DKR"], np.float32)
    W_O = np.asarray(inputs["W_O"], np.float32)
    qw = np.asarray(inputs["q_norm_w"], np.float32)
    kvw = np.asarray(inputs["kv_norm_w"], np.float32)

    maskp = (mask[0:512, 0:512] / SCALE).astype(BF)
    cos4 = np.tile(np.ascontiguousarray(pos_cos.T), (4, 1)).astype(np.float32)
    sin4 = np.tile(np.ascontiguousarray(pos_sin.T), (4, 1)).astype(np.float32)
    wdkr = np.ascontiguousarray(
        np.concatenate([W_DKR[:, 0::2], W_DKR[:, 1::2]], axis=1))
    wuq_n = W_UQ * qw[:, None]
    wuqr_n = (W_UQR * qw[:, None]).reshape(DCQ, H, ROPE)
    wuk_n = W_UK * kvw[:, None]
    wuv_n = W_UV * kvw[:, None]
    ident = np.eye(128, dtype=np.float32).astype(BF)
    ones_r = np.ones((1, 128), np.float32)
    ones_c = np.ones((128, 1), np.float32)

    in_maps = []
    for c in range(NCORES):
        b, g = divmod(c, 4)
        hs = slice(g * HL * NOPE, (g + 1) * HL * NOPE)
        heads = list(range(g * HL, (g + 1) * HL))
        wuqre = np.concatenate([wuqr_n[:, h, 0::2] for h in heads], axis=1)
        wuqro = np.concatenate([wuqr_n[:, h, 1::2] for h in heads], axis=1)
        in_maps.append({
            "xT": np.ascontiguousarray(x[b].T),
            "maskp": maskp,
            "cos4": cos4,
            "sin4": sin4,
            "wdq": W_DQ,
            "wdkv": W_DKV,
            "wdkr": wdkr,
            "wuq": np.ascontiguousarray(wuq_n[:, hs]),
            "wuqre": np.ascontiguousarray(wuqre),
            "wuqro": np.ascontiguousarray(wuqro),
            "wuk": np.ascontiguousarray(wuk_n[:, hs]),
            "wuv": np.ascontiguousarray(wuv_n[:, hs]),
            "wo4": np.ascontiguousarray(W_O[hs, :]).astype(BF),
            "ident": ident,
            "ones_r": ones_r,
            "ones_c": ones_c,
        })
    return in_maps


def kernel(**inputs):
    from concourse.bass_utils import run_bass_kernel_spmd

    if "nc" not in _BUILD_CACHE:
        _BUILD_CACHE["nc"] = build_nc()
    nc = _BUILD_CACHE["nc"]
    in_maps = _shard(inputs)
    res = run_bass_kernel_spmd(nc, in_maps, core_ids=list(range(NCORES)))
    out = np.zeros((B, S, D), np.float32)
    for c in range(NCORES):
        out[c // 4] += np.asarray(res.results[c]["outp"], np.float32)
    return out
